# revision 32
# baseline (speedup 1.0000x reference)
"""AttnBlock (GroupNorm + 1x1-conv QKV self-attention + proj + residual) on 8 trn2 cores.

Sharding: data-parallel over (batch, q-half): core = 2*b + half. Each core gets
x[b] spatially rolled so its 2048 query positions are always columns 0:2048
(attention/GroupNorm are permutation-invariant over positions, 1x1 convs are
pointwise, so rolling is exact). Full K/V are computed redundantly per pair.

Device pipeline (per core, c=512, hw=4096, P=128):
  x [512,4096] f32 -> GroupNorm (bn_stats + tiny mask-matmuls for the 16-chan
  group combine/broadcast) -> hn bf16.
  QKV in bf16: k [c,4096], q [c,2048] (c-major), vT [kpos,c] (computed
  transposed directly: lhsT=hn-slice, rhs=wvT).
  Attention per q-block of 512: scores^T [kpos,qpos] = lhsT(k-slice)@q,
  exp on ACT (no max subtraction: |s|<~8 for these inputs), softmax denom l
  via ones-matmul, att0[c,q] = lhsT(vT-slice)@attn^T accumulated over kpos,
  1/l broadcast via rank-1 matmul, proj with wpT, +bias'+residual.
  bv/bp are folded host-side: out = x + wp@att0*(1/l) + (bp + wp@bv).
"""

import numpy as np

B, C, HW = 4, 512, 64 * 64
HALF = HW // 2            # 2048 query positions per core
P = 128
NCT = C // P              # 4 channel part-tiles
NKT = HW // P             # 32 kpos tiles
NQB = HALF // 512         # 4 q-blocks of 512
NG_TILE = P // 16         # 8 groups per part-tile
EPS = 1e-6
QKS = 4.0                 # q/k pre-scale: keeps fp8 values out of subnormals
SCALE = float(C) ** -0.5 / (QKS * QKS)

_CACHE = {}


def _f32r(ap):
    from concourse import mybir
    return ap.bitcast(mybir.dt.float32r)


def _build():
    import concourse.bacc as bacc
    import concourse.tile as tile
    from concourse import mybir

    f32 = mybir.dt.float32
    bf16 = mybir.dt.bfloat16
    AF = mybir.ActivationFunctionType
    ALU = mybir.AluOpType

    nc = bacc.Bacc(
        "TRN2",
        target_bir_lowering=False,
        debug=False,
        enable_asserts=False,
        num_devices=8,
    )

    f8 = mybir.dt.float8e4
    DR = mybir.MatmulPerfMode.DoubleRow

    x_d = nc.dram_tensor("x", [C, HW], f32, kind="ExternalInput")
    wq8_d = nc.dram_tensor("wq8", [2, P, 2, C], f8, kind="ExternalInput")
    wk8_d = nc.dram_tensor("wk8", [2, P, 2, C], f8, kind="ExternalInput")
    wv_d = nc.dram_tensor("wvt", [C, C], bf16, kind="ExternalInput")
    wp_d = nc.dram_tensor("wpt", [C, C], bf16, kind="ExternalInput")
    bq_d = nc.dram_tensor("bq", [C, 1], f32, kind="ExternalInput")
    bk_d = nc.dram_tensor("bk", [C, 1], f32, kind="ExternalInput")
    bp_d = nc.dram_tensor("bpp", [C, 1], f32, kind="ExternalInput")
    gnw_d = nc.dram_tensor("gnw", [C, 1], f32, kind="ExternalInput")
    gnb_d = nc.dram_tensor("gnb", [C, 1], f32, kind="ExternalInput")
    m1_d = nc.dram_tensor("mask1", [P, NG_TILE], f32, kind="ExternalInput")
    m2_d = nc.dram_tensor("mask2", [NG_TILE, P], f32, kind="ExternalInput")
    ones_d = nc.dram_tensor("onesf", [P, P], f32, kind="ExternalInput")
    onesb_d = nc.dram_tensor("onesb", [P, 1], bf16, kind="ExternalInput")
    out_d = nc.dram_tensor("out", [C, HALF], f32, kind="ExternalOutput")

    with tile.TileContext(nc) as tc:
        with (
            tc.tile_pool(name="pw", bufs=1) as pw,
            tc.tile_pool(name="pc", bufs=1) as pconst,
            tc.tile_pool(name="pact", bufs=1) as pact,
            tc.tile_pool(name="pmisc", bufs=3) as pmisc,
            tc.tile_pool(name="ppsA", bufs=2, space="PSUM") as pps,
        ):
            # ---- x loads first (phase A is gated on them), split across
            # HWDGE (sync) and SWDGE (gpsimd) queues for aggregate bandwidth ----
            pxs_cm = tc.tile_pool(name="pxs", bufs=1)
            pxs = pxs_cm.__enter__()
            xs = []
            for i in range(NCT):
                t = pxs.tile([P, HW], f32, name=f"xs{i}", tag=f"xs{i}")
                for ch in range(4):
                    eng = nc.sync if (i * 4 + ch) % 2 == 0 else nc.gpsimd
                    eng.dma_start(
                        out=t[:, ch * 1024:(ch + 1) * 1024],
                        in_=x_d[i * P:(i + 1) * P, ch * 1024:(ch + 1) * 1024])
                xs.append(t)

            # ---- constants / weights ----
            w_sb = {}
            for nm, dt_ in (("wv", wv_d), ("wp", wp_d)):
                for ci in range(NCT):
                    t = pw.tile([P, C], bf16, name=f"{nm}{ci}", tag=f"{nm}{ci}")
                    nc.sync.dma_start(out=t, in_=dt_[ci * P:(ci + 1) * P, :])
                    w_sb[nm, ci] = t
            wq8s, wk8s = [], []
            for nm, dt_, lst in (("wq8", wq8_d, wq8s), ("wk8", wk8_d, wk8s)):
                for g in range(2):
                    t = pw.tile([P, 2, C], f8, name=f"{nm}_{g}", tag=f"{nm}_{g}")
                    nc.sync.dma_start(out=t, in_=dt_[g, :, :, :])
                    lst.append(t)
            m1 = pconst.tile([P, NG_TILE], f32, name="m1", tag="m1")
            nc.sync.dma_start(out=m1, in_=m1_d[:, :])
            m2 = pconst.tile([NG_TILE, P], f32, name="m2", tag="m2")
            nc.sync.dma_start(out=m2, in_=m2_d[:, :])
            ones = pconst.tile([P, P], f32, name="ones", tag="ones")
            nc.sync.dma_start(out=ones, in_=ones_d[:, :])
            onesb = pconst.tile([P, 1], bf16, name="onesb", tag="onesb")
            nc.sync.dma_start(out=onesb, in_=onesb_d[:, :])
            eps_col = pconst.tile([P, 1], f32, name="eps", tag="eps")
            nc.vector.memset(eps_col, EPS)
            cols = {}
            for nm, dt_ in (("bq", bq_d), ("bk", bk_d), ("bp", bp_d),
                            ("gnw", gnw_d), ("gnb", gnb_d)):
                for ci in range(NCT):
                    t = pconst.tile([P, 1], f32, name=f"{nm}{ci}", tag=f"{nm}{ci}")
                    nc.sync.dma_start(out=t, in_=dt_[ci * P:(ci + 1) * P, :])
                    cols[nm, ci] = t

            hn = [pact.tile([P, HW], bf16, name=f"hn{i}", tag=f"hn{i}") for i in range(NCT)]
            hn8 = [pact.tile([P, 2, HW], f8, name=f"hn8_{g}", tag=f"hn8_{g}") for g in range(2)]
            k8 = [pact.tile([P, 2, HW], f8, name=f"k8_{g}", tag=f"k8_{g}") for g in range(2)]
            q8 = [pact.tile([P, 2, HALF], f8, name=f"q8_{g}", tag=f"q8_{g}") for g in range(2)]
            vt = [pact.tile([P, C], bf16, name=f"vt{t}", tag=f"vt{t}") for t in range(NKT)]

            # ---- phase A: GroupNorm, cast to bf16/fp8 ----
            with (
                tc.tile_pool(name="ppgn", bufs=1, space="PSUM") as pgn,
            ):
                for i in range(NCT):
                    st6 = pmisc.tile([P, 8, 6], f32, name="st6", tag="st6")
                    for sg in range(8):
                        nc.vector.bn_stats(out=st6[:, sg, :],
                                           in_=xs[i][:, sg * 512:(sg + 1) * 512])
                    mv = pmisc.tile([P, 2], f32, name="mv", tag="mv")
                    nc.vector.bn_aggr(out=mv, in_=st6)
                    # st2 = (mean, E[x^2]) per channel
                    msq = pmisc.tile([P, 1], f32, name="msq", tag="msq")
                    nc.gpsimd.tensor_mul(out=msq, in0=mv[:, 0:1], in1=mv[:, 0:1])
                    st2 = pmisc.tile([P, 2], f32, name="st2", tag="st2")
                    nc.gpsimd.tensor_copy(out=st2[:, 0:1], in_=mv[:, 0:1])
                    nc.gpsimd.tensor_add(out=st2[:, 1:2], in0=mv[:, 1:2], in1=msq)
                    # group combine: [8,2] = mask1.T @ st2
                    pg = pgn.tile([NG_TILE, 2], f32, name="pg", tag="pg")
                    nc.tensor.matmul(out=pg, lhsT=m1, rhs=st2, start=True, stop=True)
                    gsb = pmisc.tile([NG_TILE, 2], f32, name="gsb", tag="gsb")
                    nc.vector.tensor_copy(out=gsb, in_=pg)
                    gm2 = pmisc.tile([NG_TILE, 1], f32, name="gm2", tag="gm2")
                    nc.gpsimd.tensor_mul(out=gm2, in0=gsb[:, 0:1], in1=gsb[:, 0:1])
                    gvar = pmisc.tile([NG_TILE, 1], f32, name="gvar", tag="gvar")
                    nc.gpsimd.tensor_tensor(out=gvar, in0=gsb[:, 1:2], in1=gm2,
                                            op=ALU.subtract)
                    gstd = pmisc.tile([NG_TILE, 1], f32, name="gstd", tag="gstd")
                    nc.scalar.activation(out=gstd, in_=gvar, func=AF.Sqrt,
                                         bias=eps_col[0:NG_TILE, :], scale=1.0)
                    gr2 = pmisc.tile([NG_TILE, 2], f32, name="gr2", tag="gr2")
                    nc.gpsimd.tensor_copy(out=gr2[:, 0:1], in_=gsb[:, 0:1])
                    nc.vector.reciprocal(out=gr2[:, 1:2], in_=gstd)
                    # broadcast back to channels: [128,2] = mask2.T(one-hot) @ gr2
                    pb = pgn.tile([P, 2], f32, name="pb", tag="pb")
                    nc.tensor.matmul(out=pb, lhsT=m2, rhs=gr2, start=True, stop=True)
                    mr = pmisc.tile([P, 2], f32, name="mr", tag="mr")
                    nc.vector.tensor_copy(out=mr, in_=pb)
                    sc = pmisc.tile([P, 1], f32, name="sc", tag="sc")
                    nc.gpsimd.tensor_mul(out=sc, in0=mr[:, 1:2], in1=cols["gnw", i])
                    tmpb = pmisc.tile([P, 1], f32, name="tmpb", tag="tmpb")
                    nc.gpsimd.tensor_mul(out=tmpb, in0=mr[:, 0:1], in1=sc)
                    bc = pmisc.tile([P, 1], f32, name="bc", tag="bc")
                    nc.gpsimd.tensor_tensor(out=bc, in0=cols["gnb", i], in1=tmpb,
                                            op=ALU.subtract)
                    # fused normalize+cast: hn = xs*sc + bc in both dtypes
                    # (bf16 on DVE for v-path, fp8 on ACT for q/k-path)
                    nc.vector.tensor_scalar(out=hn[i], in0=xs[i],
                                            scalar1=sc, scalar2=bc,
                                            op0=ALU.mult, op1=ALU.add)
                    nc.scalar.activation(out=hn8[i // 2][:, i % 2, :], in_=xs[i],
                                         func=AF.Identity, bias=bc, scale=sc)

            pxs_cm.__exit__(None, None, None)  # free xs SBUF before phase B/C

            # ---- phase B: QKV projections (q/k fp8 DoubleRow, v bf16) ----
            # Interleave v-proj (PE-heavy, DVE copyback) with k/q-proj
            # (PE-light, ACT copyback) so PE stays busy during ACT copies.
            def kq_proj(w8s, m, nb, dst, bias):
                ps = pps.tile([P, 512], f32, name="ps", tag="ps")
                for g in range(2):
                    nc.tensor.matmul(
                        out=ps,
                        lhsT=w8s[g][:, :, m * P:(m + 1) * P],
                        rhs=hn8[g][:, :, nb * 512:(nb + 1) * 512],
                        start=(g == 0), stop=(g == 1), perf_mode=DR)
                nc.scalar.activation(out=dst[m // 2][:, m % 2, nb * 512:(nb + 1) * 512],
                                     in_=ps, func=AF.Identity,
                                     bias=bias, scale=1.0)

            with tc.tile_pool(name="ppsB", bufs=2, space="PSUM") as ppsB:
                def v_proj(kt):
                    ps = ppsB.tile([P, 512], f32, name="psv", tag="psv")
                    for ci in range(NCT):
                        nc.tensor.matmul(
                            out=ps,
                            lhsT=hn[ci][:, kt * P:(kt + 1) * P],
                            rhs=w_sb["wv", ci],
                            start=(ci == 0), stop=(ci == NCT - 1))
                    nc.vector.tensor_copy(out=vt[kt], in_=ps)

                for m in range(NCT):
                    for nb in range(HW // 512):
                        kq_proj(wk8s, m, nb, k8, cols["bk", m])
                        v_proj(m * 8 + nb)
                        if nb < HALF // 512:
                            kq_proj(wq8s, m, nb, q8, cols["bq", m])

            # ---- phase C: attention + proj + residual, per q-block ----
            with (
                tc.tile_pool(name="pat", bufs=4) as pat,
                tc.tile_pool(name="patt", bufs=3) as patt,
                tc.tile_pool(name="pxr", bufs=3) as pxr,
                tc.tile_pool(name="pout", bufs=6) as pout,
                tc.tile_pool(name="pwb", bufs=2) as pwb,
                tc.tile_pool(name="ppo", bufs=1, space="PSUM") as ppo,
                tc.tile_pool(name="ppm", bufs=2, space="PSUM") as ppm,
            ):
                for qb in range(NQB):
                    qlo = qb * 512
                    pl = ppm.tile([1, 512], f32, name="pl", tag="pm")
                    po = [ppo.tile([P, 512], f32, name=f"po{cm}", tag=f"po{cm}") for cm in range(NCT)]

                    # software pipeline: PE issues scores(kt+1) before the
                    # l/att0 matmuls of kt, hiding the exp(kt) ACT latency
                    def consume(at, kt):
                        nc.tensor.matmul(out=pl, lhsT=onesb, rhs=at,
                                         start=(kt == 0), stop=(kt == NKT - 1),
                                         skip_group_check=True)
                        for cm in range(NCT):
                            nc.tensor.matmul(
                                out=po[cm],
                                lhsT=vt[kt][:, cm * P:(cm + 1) * P],
                                rhs=at,
                                start=(kt == 0), stop=(kt == NKT - 1),
                                skip_group_check=True)

                    at_prev = None
                    for kt in range(NKT):
                        ps = pps.tile([P, 512], f32, name="ps", tag="ps")
                        for g in range(2):
                            nc.tensor.matmul(
                                out=ps,
                                lhsT=k8[g][:, :, kt * P:(kt + 1) * P],
                                rhs=q8[g][:, :, qlo:qlo + 512],
                                start=(g == 0), stop=(g == 1), perf_mode=DR)
                        at = pat.tile([P, 512], bf16, name="attnT", tag="attnT")
                        nc.scalar.activation(out=at, in_=ps, func=AF.Exp,
                                             scale=SCALE)
                        if at_prev is not None:
                            consume(at_prev, kt - 1)
                        at_prev = at
                    consume(at_prev, NKT - 1)
                    # softmax denominators -> broadcast 1/l to all partitions
                    wrow = pmisc.tile([1, 512], f32, name="wrow", tag="wrow")
                    nc.vector.reciprocal(out=wrow, in_=pl)
                    pwbc = ppm.tile([P, 512], f32, name="pwbc", tag="pm")
                    nc.tensor.matmul(out=pwbc, lhsT=ones[0:1, :], rhs=wrow,
                                     start=True, stop=True)
                    wbc = pwb.tile([P, 512], f32, name="wbc", tag="wbc")
                    nc.vector.tensor_copy(out=wbc, in_=pwbc)
                    att = []
                    for cm in range(NCT):
                        a = patt.tile([P, 512], bf16, name=f"att{cm}", tag=f"att{cm}")
                        nc.vector.tensor_mul(out=a, in0=po[cm], in1=wbc)
                        att.append(a)
                    for om in range(NCT):
                        pp = ppm.tile([P, 512], f32, name=f"pp{om}", tag="pm")
                        for m in range(NCT):
                            nc.tensor.matmul(
                                out=pp,
                                lhsT=w_sb["wp", m][:, om * P:(om + 1) * P],
                                rhs=att[m],
                                start=(m == 0), stop=(m == NCT - 1))
                        ob = pout.tile([P, 512], f32, name="outsb", tag="outsb")
                        nc.scalar.activation(out=ob, in_=pp, func=AF.Identity,
                                             bias=cols["bp", om], scale=1.0)
                        xr = pxr.tile([P, 512], f32, name=f"xr{om}", tag=f"xr{om}")
                        nc.sync.dma_start(
                            out=xr, in_=x_d[om * P:(om + 1) * P, qlo:qlo + 512])
                        nc.vector.tensor_add(out=ob, in0=ob, in1=xr)
                        nc.sync.dma_start(
                            out=out_d[om * P:(om + 1) * P, qlo:qlo + 512], in_=ob)

    nc.compile()
    return nc


def _get_nc():
    if "nc" not in _CACHE:
        _CACHE["nc"] = _build()
    return _CACHE["nc"]


def _make_in_maps(x, gn_scale, gn_bias, wq, bq, wk, bk, wv, bv, wp, bp):
    import ml_dtypes
    bf16 = ml_dtypes.bfloat16
    f8 = ml_dtypes.float8_e4m3

    def interleave8(w, s=1.0):
        # wT[c_in, c_out] -> [g, ki, ko, c_out] with c_in = 256*g + 128*ko + ki
        wT = np.asarray(w, np.float32).T * s
        return np.ascontiguousarray(
            wT.reshape(2, 2, P, C).transpose(0, 2, 1, 3)).astype(f8)

    xf = np.asarray(x, np.float32).reshape(B, C, HW)
    shared = {
        "wq8": interleave8(wq, QKS),
        "wk8": interleave8(wk, QKS),
        "wvt": np.ascontiguousarray(np.asarray(wv, np.float32).T).astype(bf16),
        "wpt": np.ascontiguousarray(np.asarray(wp, np.float32).T).astype(bf16),
        "bq": np.asarray(bq, np.float32).reshape(C, 1) * QKS,
        "bk": np.asarray(bk, np.float32).reshape(C, 1) * QKS,
        # fold v/proj biases: out = x + wp@att0/l + (bp + wp@bv)
        "bpp": (np.asarray(bp, np.float32)
                + np.asarray(wp, np.float32) @ np.asarray(bv, np.float32)
                ).reshape(C, 1),
        "gnw": np.asarray(gn_scale, np.float32).reshape(C, 1),
        "gnb": np.asarray(gn_bias, np.float32).reshape(C, 1),
        "mask1": (np.eye(NG_TILE, dtype=np.float32) / 16.0
                  ).repeat(16, axis=0).reshape(P, NG_TILE),
        "mask2": np.eye(NG_TILE, dtype=np.float32
                        ).repeat(16, axis=1).reshape(NG_TILE, P),
        "onesf": np.ones((P, P), np.float32),
        "onesb": np.ones((P, 1), np.float32).astype(bf16),
    }
    in_maps = []
    for core in range(8):
        b_idx, half = divmod(core, 2)
        xb = xf[b_idx]
        if half:
            xb = np.concatenate([xb[:, HALF:], xb[:, :HALF]], axis=1)
        in_maps.append({"x": np.ascontiguousarray(xb), **shared})
    return in_maps


def _run(inputs, trace=False):
    from concourse.bass_utils import run_bass_kernel_spmd

    nc = _get_nc()
    in_maps = _make_in_maps(**inputs)
    res = run_bass_kernel_spmd(nc, in_maps, core_ids=list(range(8)), trace=trace)
    out = np.empty((B, C, HW), np.float32)
    for core in range(8):
        b_idx, half = divmod(core, 2)
        out[b_idx][:, half * HALF:(half + 1) * HALF] = res.results[core]["out"]
    return out.reshape(B, C, 64, 64), res


def kernel(**inputs):
    out, _ = _run(inputs, trace=False)
    return out


# revision 35
# speedup vs baseline: 1.0014x; 1.0014x over previous
"""AttnBlock (GroupNorm + 1x1-conv QKV self-attention + proj + residual) on 8 trn2 cores.

Sharding: data-parallel over (batch, q-half): core = 2*b + half. Each core gets
x[b] spatially rolled so its 2048 query positions are always columns 0:2048
(attention/GroupNorm are permutation-invariant over positions, 1x1 convs are
pointwise, so rolling is exact). Full K/V are computed redundantly per pair.

Device pipeline (per core, c=512, hw=4096, P=128):
  x [512,4096] f32 -> GroupNorm (bn_stats + tiny mask-matmuls for the 16-chan
  group combine/broadcast) -> hn bf16.
  QKV in bf16: k [c,4096], q [c,2048] (c-major), vT [kpos,c] (computed
  transposed directly: lhsT=hn-slice, rhs=wvT).
  Attention per q-block of 512: scores^T [kpos,qpos] = lhsT(k-slice)@q,
  exp on ACT (no max subtraction: |s|<~8 for these inputs), softmax denom l
  via ones-matmul, att0[c,q] = lhsT(vT-slice)@attn^T accumulated over kpos,
  1/l broadcast via rank-1 matmul, proj with wpT, +bias'+residual.
  bv/bp are folded host-side: out = x + wp@att0*(1/l) + (bp + wp@bv).
"""

import numpy as np

B, C, HW = 4, 512, 64 * 64
HALF = HW // 2            # 2048 query positions per core
P = 128
NCT = C // P              # 4 channel part-tiles
NKT = HW // P             # 32 kpos tiles
NQB = HALF // 512         # 4 q-blocks of 512
NG_TILE = P // 16         # 8 groups per part-tile
EPS = 1e-6
QKS = 4.0                 # q/k pre-scale: keeps fp8 values out of subnormals
SCALE = float(C) ** -0.5 / (QKS * QKS)

_CACHE = {}


def _f32r(ap):
    from concourse import mybir
    return ap.bitcast(mybir.dt.float32r)


def _build():
    import concourse.bacc as bacc
    import concourse.tile as tile
    from concourse import mybir

    f32 = mybir.dt.float32
    bf16 = mybir.dt.bfloat16
    AF = mybir.ActivationFunctionType
    ALU = mybir.AluOpType

    nc = bacc.Bacc(
        "TRN2",
        target_bir_lowering=False,
        debug=False,
        enable_asserts=False,
        num_devices=8,
    )

    f8 = mybir.dt.float8e4
    DR = mybir.MatmulPerfMode.DoubleRow

    x_d = nc.dram_tensor("x", [C, HW], f32, kind="ExternalInput")
    wq8_d = nc.dram_tensor("wq8", [2, P, 2, C], f8, kind="ExternalInput")
    wk8_d = nc.dram_tensor("wk8", [2, P, 2, C], f8, kind="ExternalInput")
    wv_d = nc.dram_tensor("wvt", [C, C], bf16, kind="ExternalInput")
    wp_d = nc.dram_tensor("wpt", [C, C], bf16, kind="ExternalInput")
    bq_d = nc.dram_tensor("bq", [C, 1], f32, kind="ExternalInput")
    bk_d = nc.dram_tensor("bk", [C, 1], f32, kind="ExternalInput")
    bp_d = nc.dram_tensor("bpp", [C, 1], f32, kind="ExternalInput")
    gnw_d = nc.dram_tensor("gnw", [C, 1], f32, kind="ExternalInput")
    gnb_d = nc.dram_tensor("gnb", [C, 1], f32, kind="ExternalInput")
    m1_d = nc.dram_tensor("mask1", [P, NG_TILE], f32, kind="ExternalInput")
    m2_d = nc.dram_tensor("mask2", [NG_TILE, P], f32, kind="ExternalInput")
    ones_d = nc.dram_tensor("onesf", [P, P], f32, kind="ExternalInput")
    onesb_d = nc.dram_tensor("onesb", [P, 1], bf16, kind="ExternalInput")
    out_d = nc.dram_tensor("out", [C, HALF], f32, kind="ExternalOutput")

    with tile.TileContext(nc) as tc:
        with (
            tc.tile_pool(name="pw", bufs=1) as pw,
            tc.tile_pool(name="pc", bufs=1) as pconst,
            tc.tile_pool(name="pact", bufs=1) as pact,
            tc.tile_pool(name="pmisc", bufs=3) as pmisc,
            tc.tile_pool(name="ppsA", bufs=2, space="PSUM") as pps,
        ):
            # ---- x loads first (phase A is gated on them), split across
            # HWDGE (sync) and SWDGE (gpsimd) queues for aggregate bandwidth ----
            pxs_cm = tc.tile_pool(name="pxs", bufs=1)
            pxs = pxs_cm.__enter__()
            xs = []
            for i in range(NCT):
                t = pxs.tile([P, HW], f32, name=f"xs{i}", tag=f"xs{i}")
                for ch in range(4):
                    eng = nc.sync if (i * 4 + ch) % 2 == 0 else nc.gpsimd
                    eng.dma_start(
                        out=t[:, ch * 1024:(ch + 1) * 1024],
                        in_=x_d[i * P:(i + 1) * P, ch * 1024:(ch + 1) * 1024])
                xs.append(t)

            # ---- constants / weights ----
            w_sb = {}
            for nm, dt_ in (("wv", wv_d), ("wp", wp_d)):
                for ci in range(NCT):
                    t = pw.tile([P, C], bf16, name=f"{nm}{ci}", tag=f"{nm}{ci}")
                    nc.sync.dma_start(out=t, in_=dt_[ci * P:(ci + 1) * P, :])
                    w_sb[nm, ci] = t
            wq8s, wk8s = [], []
            for nm, dt_, lst in (("wq8", wq8_d, wq8s), ("wk8", wk8_d, wk8s)):
                for g in range(2):
                    t = pw.tile([P, 2, C], f8, name=f"{nm}_{g}", tag=f"{nm}_{g}")
                    nc.sync.dma_start(out=t, in_=dt_[g, :, :, :])
                    lst.append(t)
            m1 = pconst.tile([P, NG_TILE], f32, name="m1", tag="m1")
            nc.sync.dma_start(out=m1, in_=m1_d[:, :])
            m2 = pconst.tile([NG_TILE, P], f32, name="m2", tag="m2")
            nc.sync.dma_start(out=m2, in_=m2_d[:, :])
            ones = pconst.tile([P, P], f32, name="ones", tag="ones")
            nc.sync.dma_start(out=ones, in_=ones_d[:, :])
            onesb = pconst.tile([P, 1], bf16, name="onesb", tag="onesb")
            nc.sync.dma_start(out=onesb, in_=onesb_d[:, :])
            eps_col = pconst.tile([P, 1], f32, name="eps", tag="eps")
            nc.vector.memset(eps_col, EPS)
            cols = {}
            for nm, dt_ in (("bq", bq_d), ("bk", bk_d), ("bp", bp_d),
                            ("gnw", gnw_d), ("gnb", gnb_d)):
                for ci in range(NCT):
                    t = pconst.tile([P, 1], f32, name=f"{nm}{ci}", tag=f"{nm}{ci}")
                    nc.sync.dma_start(out=t, in_=dt_[ci * P:(ci + 1) * P, :])
                    cols[nm, ci] = t

            hn = [pact.tile([P, HW], bf16, name=f"hn{i}", tag=f"hn{i}") for i in range(NCT)]
            hn8 = [pact.tile([P, 2, HW], f8, name=f"hn8_{g}", tag=f"hn8_{g}") for g in range(2)]
            k8 = [pact.tile([P, 2, HW], f8, name=f"k8_{g}", tag=f"k8_{g}") for g in range(2)]
            q8 = [pact.tile([P, 2, HALF], f8, name=f"q8_{g}", tag=f"q8_{g}") for g in range(2)]
            vt = [pact.tile([P, C], bf16, name=f"vt{t}", tag=f"vt{t}") for t in range(NKT)]

            # ---- phase A: GroupNorm, cast to bf16/fp8 ----
            with (
                tc.tile_pool(name="ppgn", bufs=1, space="PSUM") as pgn,
            ):
                for i in range(NCT):
                    st6 = pmisc.tile([P, 8, 6], f32, name="st6", tag="st6")
                    for sg in range(8):
                        nc.vector.bn_stats(out=st6[:, sg, :],
                                           in_=xs[i][:, sg * 512:(sg + 1) * 512])
                    mv = pmisc.tile([P, 2], f32, name="mv", tag="mv")
                    nc.vector.bn_aggr(out=mv, in_=st6)
                    # st2 = (mean, E[x^2]) per channel
                    msq = pmisc.tile([P, 1], f32, name="msq", tag="msq")
                    nc.gpsimd.tensor_mul(out=msq, in0=mv[:, 0:1], in1=mv[:, 0:1])
                    st2 = pmisc.tile([P, 2], f32, name="st2", tag="st2")
                    nc.gpsimd.tensor_copy(out=st2[:, 0:1], in_=mv[:, 0:1])
                    nc.gpsimd.tensor_add(out=st2[:, 1:2], in0=mv[:, 1:2], in1=msq)
                    # group combine: [8,2] = mask1.T @ st2
                    pg = pgn.tile([NG_TILE, 2], f32, name="pg", tag="pg")
                    nc.tensor.matmul(out=pg, lhsT=m1, rhs=st2, start=True, stop=True)
                    gsb = pmisc.tile([NG_TILE, 2], f32, name="gsb", tag="gsb")
                    nc.vector.tensor_copy(out=gsb, in_=pg)
                    gm2 = pmisc.tile([NG_TILE, 1], f32, name="gm2", tag="gm2")
                    nc.gpsimd.tensor_mul(out=gm2, in0=gsb[:, 0:1], in1=gsb[:, 0:1])
                    gvar = pmisc.tile([NG_TILE, 1], f32, name="gvar", tag="gvar")
                    nc.gpsimd.tensor_tensor(out=gvar, in0=gsb[:, 1:2], in1=gm2,
                                            op=ALU.subtract)
                    gstd = pmisc.tile([NG_TILE, 1], f32, name="gstd", tag="gstd")
                    nc.scalar.activation(out=gstd, in_=gvar, func=AF.Sqrt,
                                         bias=eps_col[0:NG_TILE, :], scale=1.0)
                    gr2 = pmisc.tile([NG_TILE, 2], f32, name="gr2", tag="gr2")
                    nc.gpsimd.tensor_copy(out=gr2[:, 0:1], in_=gsb[:, 0:1])
                    nc.vector.reciprocal(out=gr2[:, 1:2], in_=gstd)
                    # broadcast back to channels: [128,2] = mask2.T(one-hot) @ gr2
                    pb = pgn.tile([P, 2], f32, name="pb", tag="pb")
                    nc.tensor.matmul(out=pb, lhsT=m2, rhs=gr2, start=True, stop=True)
                    mr = pmisc.tile([P, 2], f32, name="mr", tag="mr")
                    nc.vector.tensor_copy(out=mr, in_=pb)
                    sc = pmisc.tile([P, 1], f32, name="sc", tag="sc")
                    nc.gpsimd.tensor_mul(out=sc, in0=mr[:, 1:2], in1=cols["gnw", i])
                    tmpb = pmisc.tile([P, 1], f32, name="tmpb", tag="tmpb")
                    nc.gpsimd.tensor_mul(out=tmpb, in0=mr[:, 0:1], in1=sc)
                    bc = pmisc.tile([P, 1], f32, name="bc", tag="bc")
                    nc.gpsimd.tensor_tensor(out=bc, in0=cols["gnb", i], in1=tmpb,
                                            op=ALU.subtract)
                    # fused normalize+cast: hn = xs*sc + bc in both dtypes
                    # (bf16 on DVE for v-path, fp8 on ACT for q/k-path)
                    nc.vector.tensor_scalar(out=hn[i], in0=xs[i],
                                            scalar1=sc, scalar2=bc,
                                            op0=ALU.mult, op1=ALU.add)
                    nc.scalar.activation(out=hn8[i // 2][:, i % 2, :], in_=xs[i],
                                         func=AF.Identity, bias=bc, scale=sc)

            pxs_cm.__exit__(None, None, None)  # free xs SBUF before phase B/C

            # ---- phase B: QKV projections (q/k fp8 DoubleRow, v bf16) ----
            # Interleave v-proj (PE-heavy, DVE copyback) with k/q-proj
            # (PE-light, ACT copyback) so PE stays busy during ACT copies.
            def kq_proj(w8s, m, nb, dst, bias):
                ps = pps.tile([P, 512], f32, name="ps", tag="ps")
                for g in range(2):
                    nc.tensor.matmul(
                        out=ps,
                        lhsT=w8s[g][:, :, m * P:(m + 1) * P],
                        rhs=hn8[g][:, :, nb * 512:(nb + 1) * 512],
                        start=(g == 0), stop=(g == 1), perf_mode=DR)
                nc.scalar.activation(out=dst[m // 2][:, m % 2, nb * 512:(nb + 1) * 512],
                                     in_=ps, func=AF.Identity,
                                     bias=bias, scale=1.0)

            with tc.tile_pool(name="ppsB", bufs=2, space="PSUM") as ppsB:
                def v_proj(kt):
                    ps = ppsB.tile([P, 512], f32, name="psv", tag="psv")
                    for ci in range(NCT):
                        nc.tensor.matmul(
                            out=ps,
                            lhsT=hn[ci][:, kt * P:(kt + 1) * P],
                            rhs=w_sb["wv", ci],
                            start=(ci == 0), stop=(ci == NCT - 1))
                    nc.vector.tensor_copy(out=vt[kt], in_=ps)

                for m in range(NCT):
                    for nb in range(HW // 512):
                        kq_proj(wk8s, m, nb, k8, cols["bk", m])
                        v_proj(m * 8 + nb)
                        if nb < HALF // 512:
                            kq_proj(wq8s, m, nb, q8, cols["bq", m])

            # ---- phase C: attention + proj + residual, per q-block ----
            with (
                tc.tile_pool(name="pat", bufs=4) as pat,
                tc.tile_pool(name="patt", bufs=3) as patt,
                tc.tile_pool(name="pxr", bufs=3) as pxr,
                tc.tile_pool(name="pout", bufs=6) as pout,
                tc.tile_pool(name="pwb", bufs=2) as pwb,
                tc.tile_pool(name="ppo", bufs=1, space="PSUM") as ppo,
                tc.tile_pool(name="ppm", bufs=2, space="PSUM") as ppm,
            ):
                def make_tail(pl, po, qlo):
                    # deferred per-block epilogue: softmax denominators,
                    # 1/l broadcast, normalize, proj, bias+residual, store
                    def tail():
                        wrow = pmisc.tile([1, 512], f32, name="wrow", tag="wrow")
                        nc.vector.reciprocal(out=wrow, in_=pl)
                        pwbc = ppm.tile([P, 512], f32, name="pwbc", tag="pm")
                        nc.tensor.matmul(out=pwbc, lhsT=ones[0:1, :], rhs=wrow,
                                         start=True, stop=True)
                        wbc = pwb.tile([P, 512], f32, name="wbc", tag="wbc")
                        nc.vector.tensor_copy(out=wbc, in_=pwbc)
                        att = []
                        for cm in range(NCT):
                            a = patt.tile([P, 512], bf16, name=f"att{cm}", tag=f"att{cm}")
                            nc.vector.tensor_mul(out=a, in0=po[cm], in1=wbc)
                            att.append(a)
                        for om in range(NCT):
                            pp = ppm.tile([P, 512], f32, name=f"pp{om}", tag="pm")
                            for m in range(NCT):
                                nc.tensor.matmul(
                                    out=pp,
                                    lhsT=w_sb["wp", m][:, om * P:(om + 1) * P],
                                    rhs=att[m],
                                    start=(m == 0), stop=(m == NCT - 1))
                            ob = pout.tile([P, 512], f32, name="outsb", tag="outsb")
                            nc.scalar.activation(out=ob, in_=pp, func=AF.Identity,
                                                 bias=cols["bp", om], scale=1.0)
                            xr = pxr.tile([P, 512], f32, name=f"xr{om}", tag=f"xr{om}")
                            nc.sync.dma_start(
                                out=xr, in_=x_d[om * P:(om + 1) * P, qlo:qlo + 512])
                            nc.vector.tensor_add(out=ob, in0=ob, in1=xr)
                            nc.sync.dma_start(
                                out=out_d[om * P:(om + 1) * P, qlo:qlo + 512], in_=ob)
                    return tail

                prev_tail = None
                for qb in range(NQB):
                    qlo = qb * 512
                    pl, po = None, None

                    # two-level software pipeline: (a) PE issues scores(kt+1)
                    # before l/att0(kt) so exp latency is hidden; (b) the
                    # previous block's tail is emitted after scores(1) so its
                    # PE work rides inside this block's stream and the PSUM
                    # slot handoff never stalls the engine queue.
                    def consume(at, kt):
                        nc.tensor.matmul(out=pl, lhsT=onesb, rhs=at,
                                         start=(kt == 0), stop=(kt == NKT - 1),
                                         skip_group_check=True)
                        for cm in range(NCT):
                            nc.tensor.matmul(
                                out=po[cm],
                                lhsT=vt[kt][:, cm * P:(cm + 1) * P],
                                rhs=at,
                                start=(kt == 0), stop=(kt == NKT - 1),
                                skip_group_check=True)

                    at_prev = None
                    for kt in range(NKT):
                        ps = pps.tile([P, 512], f32, name="ps", tag="ps")
                        for g in range(2):
                            nc.tensor.matmul(
                                out=ps,
                                lhsT=k8[g][:, :, kt * P:(kt + 1) * P],
                                rhs=q8[g][:, :, qlo:qlo + 512],
                                start=(g == 0), stop=(g == 1), perf_mode=DR)
                        at = pat.tile([P, 512], bf16, name="attnT", tag="attnT")
                        nc.scalar.activation(out=at, in_=ps, func=AF.Exp,
                                             scale=SCALE)
                        if kt == 1 and prev_tail is not None:
                            prev_tail()
                            prev_tail = None
                        if at_prev is not None:
                            if po is None:
                                pl = ppm.tile([1, 512], f32, name="pl", tag="pm")
                                po = [ppo.tile([P, 512], f32, name=f"po{cm}",
                                               tag=f"po{cm}") for cm in range(NCT)]
                            consume(at_prev, kt - 1)
                        at_prev = at
                    consume(at_prev, NKT - 1)
                    prev_tail = make_tail(pl, po, qlo)
                prev_tail()

    nc.compile()
    return nc


def _get_nc():
    if "nc" not in _CACHE:
        _CACHE["nc"] = _build()
    return _CACHE["nc"]


def _make_in_maps(x, gn_scale, gn_bias, wq, bq, wk, bk, wv, bv, wp, bp):
    import ml_dtypes
    bf16 = ml_dtypes.bfloat16
    f8 = ml_dtypes.float8_e4m3

    def interleave8(w, s=1.0):
        # wT[c_in, c_out] -> [g, ki, ko, c_out] with c_in = 256*g + 128*ko + ki
        wT = np.asarray(w, np.float32).T * s
        return np.ascontiguousarray(
            wT.reshape(2, 2, P, C).transpose(0, 2, 1, 3)).astype(f8)

    xf = np.asarray(x, np.float32).reshape(B, C, HW)
    shared = {
        "wq8": interleave8(wq, QKS),
        "wk8": interleave8(wk, QKS),
        "wvt": np.ascontiguousarray(np.asarray(wv, np.float32).T).astype(bf16),
        "wpt": np.ascontiguousarray(np.asarray(wp, np.float32).T).astype(bf16),
        "bq": np.asarray(bq, np.float32).reshape(C, 1) * QKS,
        "bk": np.asarray(bk, np.float32).reshape(C, 1) * QKS,
        # fold v/proj biases: out = x + wp@att0/l + (bp + wp@bv)
        "bpp": (np.asarray(bp, np.float32)
                + np.asarray(wp, np.float32) @ np.asarray(bv, np.float32)
                ).reshape(C, 1),
        "gnw": np.asarray(gn_scale, np.float32).reshape(C, 1),
        "gnb": np.asarray(gn_bias, np.float32).reshape(C, 1),
        "mask1": (np.eye(NG_TILE, dtype=np.float32) / 16.0
                  ).repeat(16, axis=0).reshape(P, NG_TILE),
        "mask2": np.eye(NG_TILE, dtype=np.float32
                        ).repeat(16, axis=1).reshape(NG_TILE, P),
        "onesf": np.ones((P, P), np.float32),
        "onesb": np.ones((P, 1), np.float32).astype(bf16),
    }
    in_maps = []
    for core in range(8):
        b_idx, half = divmod(core, 2)
        xb = xf[b_idx]
        if half:
            xb = np.concatenate([xb[:, HALF:], xb[:, :HALF]], axis=1)
        in_maps.append({"x": np.ascontiguousarray(xb), **shared})
    return in_maps


def _run(inputs, trace=False):
    from concourse.bass_utils import run_bass_kernel_spmd

    nc = _get_nc()
    in_maps = _make_in_maps(**inputs)
    res = run_bass_kernel_spmd(nc, in_maps, core_ids=list(range(8)), trace=trace)
    out = np.empty((B, C, HW), np.float32)
    for core in range(8):
        b_idx, half = divmod(core, 2)
        out[b_idx][:, half * HALF:(half + 1) * HALF] = res.results[core]["out"]
    return out.reshape(B, C, 64, 64), res


def kernel(**inputs):
    out, _ = _run(inputs, trace=False)
    return out
